# revision 31
# baseline (speedup 1.0000x reference)
"""Trainium2 Bass kernel for nn_MHA_65429531787938.

MHA with a faithful-quirk softmax over dim=0 (the batch axis, B=2).
For B=2 the batch-softmax collapses to an elementwise sigmoid:
    attn0 = sigmoid((s0 - s1)/SCALE),  attn1 = 1 - attn0
and (1-A0) @ V1 = colsum(V1) - A0 @ V1, so a single attention matrix
serves both batches.

Sharding: tensor-parallel over the 16 heads -> 2 heads per core
(columns of w_q/w_k/w_v, rows of W_o). Each core consumes the full x
and produces a partial output (its heads' contribution to out = vals @ W_o,
scaled by 0.25 and stored fp16); the host sums the 8 partials and
multiplies by 4.

v2 changes vs the original baseline:
  - x is transposed + cast to fp16 on the HOST ([B, D, S]); the kernel
    DMA-loads x^T tiles directly, eliminating all 288 PE transposes,
    the 32 ACT fp32->fp16 casts, and half the x HBM traffic.
  - weights are cast to fp16 on the host (no on-chip weight casts).
  - output partials are written fp16 (scaled 0.25), halving write traffic.
  - Q/K psum->sbuf copies run on the Scalar engine (idle in phase 1),
    V copies on Vector, balancing the copy load.

Per-core pipeline (heads h0=2i, h1=2i+1 -> a 128-wide slice of q/k/v dims):
  phase 1: qT/kT/vT projections from DMA-loaded x^T chunks (fp16 matmuls,
           fp32 psum); qT/kT stored batch-stacked per head ([Q0;-Q1] /
           [K0;K1]); vT -> V natural via PE transpose (V1 stored negated).
  phase 2: d^T = K0@Q0^T - K1@Q1^T in one fused matmul (contraction=128);
           A0^T = sigmoid(d^T/SCALE) on ACT (fp16 out);
           psum_av = [V0 | -V1] @ A0^T + rank-1 colsum(V1) correction.
  phase 3: out_partial = 0.25 * vals @ W_o_slice (fp16 out).
"""

import numpy as np

import concourse.bacc as bacc
import concourse.mybir as mybir
import concourse.tile as tile
from concourse import bass_utils
from concourse.masks import make_identity

B, S, D, H = 2, 2048, 1024, 16
HD = 64
SCALE = float(D) ** 0.5
NCORES = 8
HPC = H // NCORES            # heads per core = 2
MS = HPC * HD                # per-core slice width = 128
P = 128
NCH = 8                      # phase-1 chunks (B * S/512)
DT16 = mybir.dt.float16
F32 = mybir.dt.float32
OSCALE = 0.25                # fp16 partial-output scale (host multiplies by 4)


def build():
    nc = bacc.Bacc("TRN2", target_bir_lowering=False, debug=False)

    # x arrives pre-transposed+cast on host: [B, D, S] fp16
    xt_d = nc.dram_tensor("xt", [B, D, S], DT16, kind="ExternalInput").ap()
    wq_d = nc.dram_tensor("wq", [D, MS], DT16, kind="ExternalInput").ap()
    wk_d = nc.dram_tensor("wk", [D, MS], DT16, kind="ExternalInput").ap()
    wv_d = nc.dram_tensor("wv", [D, MS], DT16, kind="ExternalInput").ap()
    wo_d = nc.dram_tensor("wo", [MS, D], DT16, kind="ExternalInput").ap()
    out_d = nc.dram_tensor("out", [B, S, D], DT16, kind="ExternalOutput").ap()

    with tile.TileContext(nc) as tc:
        with tc.tile_pool(name="persist", bufs=1) as pp:
            ident16 = pp.tile([P, P], DT16, name="ident16")
            make_identity(nc, ident16[:])
            ones512 = pp.tile([1, 512], DT16)
            nc.vector.memset(ones512[:], 1.0)
            ones128 = pp.tile([P, 1], DT16)
            nc.vector.memset(ones128[:], 1.0)

            # fp16 weights straight from DRAM
            w_sb = {}
            for name, dram in (("wq", wq_d), ("wk", wk_d), ("wv", wv_d)):
                t = pp.tile([P, D // P, MS], DT16, name=f"{name}_sb")
                nc.gpsimd.dma_start(t[:], dram.rearrange("(t p) m -> p t m", p=P))
                w_sb[name] = t
            wo_sb = pp.tile([P, 2, 512], DT16)
            nc.gpsimd.dma_start(
                wo_sb[:], wo_d.rearrange("p (c n) -> p c n", c=2)
            )

            # big persistent tensors
            qsb = pp.tile([P, HPC, S], DT16)     # [(b,hd), head, qpos], b1 negated
            ksb = pp.tile([P, HPC, S], DT16)     # [(b,hd), head, kpos]
            vt_sb = pp.tile([P, B, S], DT16)     # [(h,hd), batch, kpos], b1 negated
            v_sb = pp.tile([P, S // P, HPC, B, HD], DT16)  # [k, ktile, h, b, hd]
            vals_sb = pp.tile([P, B, S], DT16)   # [(h,hd), batch, qpos]
            c1_sb = pp.tile([1, HPC, HD], DT16)  # +colsum(V1) per head

            # ---------------- phase 1: Q/K/V projections ----------------
            with tc.tile_pool(name="p1xt", bufs=6) as p1xt, \
                 tc.tile_pool(name="ps1", bufs=4, space="PSUM") as ps1, \
                 tc.tile_pool(name="ps1v", bufs=3, space="PSUM") as ps1v:
                for c in range(NCH):
                    b, j = divmod(c, NCH // B)
                    xt = p1xt.tile([P, D // P, 512], DT16, tag="xt")
                    dma_eng = nc.sync if c % 2 == 0 else nc.gpsimd
                    if c == 0:
                        # split the very first load across both rings so the
                        # pipeline fills faster
                        src = xt_d[b, :, j * 512:(j + 1) * 512].rearrange(
                            "(t p) s -> p t s", p=P)
                        nc.sync.dma_start(xt[:, :4, :], src[:, :4, :])
                        nc.gpsimd.dma_start(xt[:, 4:, :], src[:, 4:, :])
                    else:
                        dma_eng.dma_start(
                            xt[:],
                            xt_d[b, :, j * 512:(j + 1) * 512].rearrange(
                                "(t p) s -> p t s", p=P),
                        )
                    # Q/K projections -> ACT copies (batch-stacked layout)
                    for name, dest, neg in (("wq", qsb, True), ("wk", ksb, False)):
                        ps = ps1.tile([P, 512], F32, tag="proj", name="ps_p")
                        for t in range(D // P):
                            nc.tensor.matmul(
                                ps[:], w_sb[name][:, t, :], xt[:, t, :],
                                start=(t == 0), stop=(t == D // P - 1),
                            )
                        for h in range(HPC):
                            nc.scalar.mul(
                                dest[b * HD:(b + 1) * HD, h, j * 512:(j + 1) * 512],
                                ps[h * HD:(h + 1) * HD, :],
                                -1.0 if (neg and b == 1) else 1.0,
                            )
                    # V projection -> DVE copy (vT, b1 negated)
                    ps = ps1.tile([P, 512], F32, tag="proj", name="ps_p")
                    for t in range(D // P):
                        nc.tensor.matmul(
                            ps[:], w_sb["wv"][:, t, :], xt[:, t, :],
                            start=(t == 0), stop=(t == D // P - 1),
                        )
                    nc.vector.tensor_scalar_mul(
                        vt_sb[:, b, j * 512:(j + 1) * 512], ps[:],
                        -1.0 if b == 1 else 1.0,
                    )
                    # V natural layout for the 4 k-tiles of this chunk
                    pvt = ps1v.tile([P, 4, P], DT16, tag="vt", name="pvt")
                    for blk in range(4):
                        t = j * 4 + blk
                        nc.tensor.transpose(
                            pvt[:, blk, :], vt_sb[:, b, t * P:(t + 1) * P],
                            ident16[:],
                        )
                    nc.vector.tensor_copy(
                        v_sb[:, j * 4:(j + 1) * 4, :, b, :],
                        pvt[:].rearrange("p t (h d) -> p t h d", h=HPC),
                    )

            # ------------- phase 2 + 3, interleaved -------------
            with tc.tile_pool(name="p2a", bufs=8) as p2a, \
                 tc.tile_pool(name="p3o", bufs=4) as p3o, \
                 tc.tile_pool(name="ps2d", bufs=2, space="PSUM") as ps2d, \
                 tc.tile_pool(name="ps2av", bufs=2, space="PSUM") as ps2av, \
                 tc.tile_pool(name="ps3", bufs=2, space="PSUM") as ps3:
                # colsums of all (h, b) V columns; extract b=1 (stored negated)
                pc1 = ps3.tile([1, HPC * B * HD], F32, tag="o")
                for t in range(S // P):
                    nc.tensor.matmul(
                        pc1[:], ones128[:],
                        v_sb[:, t, :, :, :].rearrange("p h b d -> p (h b d)"),
                        start=(t == 0), stop=(t == S // P - 1),
                    )
                nc.vector.tensor_scalar_mul(
                    c1_sb[:],
                    pc1[:].rearrange("p (h b d) -> p h b d", h=HPC, b=B)[:, :, 1, :],
                    -1.0,
                )

                def emit_out_block(b, si, tail=False):
                    # one output-projection s-block (phase 3); fp16 out, 0.25x
                    ot = p3o.tile([P, D], DT16, tag="ot", name="ot")
                    if tail and si % 2 == 0:
                        # scores psum (pd ring) is idle in the tail: even
                        # blocks borrow a [P,1024] tile with one big ACT
                        # copy; odd blocks use the ps3 ring with DVE copies,
                        # so 4 blocks pipeline across disjoint psum + engines
                        po = ps2d.tile([P, 1024], F32, tag="d", name="pd")
                        for nch in range(2):
                            nc.tensor.matmul(
                                po[:, nch * 512:(nch + 1) * 512],
                                vals_sb[:, b, si * P:(si + 1) * P],
                                wo_sb[:, nch, :],
                                start=True, stop=True,
                            )
                        nc.scalar.mul(ot[:], po[:], OSCALE)
                        ring = nc.sync
                    elif tail:
                        for nch in range(2):
                            po = ps3.tile([P, 512], F32, tag="o", name="po")
                            nc.tensor.matmul(
                                po[:],
                                vals_sb[:, b, si * P:(si + 1) * P],
                                wo_sb[:, nch, :],
                                start=True, stop=True,
                            )
                            nc.vector.tensor_scalar_mul(
                                ot[:, nch * 512:(nch + 1) * 512], po[:], OSCALE
                            )
                        ring = nc.gpsimd
                    else:
                        for nch in range(2):
                            po = ps3.tile([P, 512], F32, tag="o", name="po")
                            nc.tensor.matmul(
                                po[:],
                                vals_sb[:, b, si * P:(si + 1) * P],
                                wo_sb[:, nch, :],
                                start=True, stop=True,
                            )
                            nc.vector.tensor_scalar_mul(
                                ot[:, nch * 512:(nch + 1) * 512], po[:], OSCALE
                            )
                        ring = nc.sync
                    ring.dma_start(out_d[b, si * P:(si + 1) * P, :], ot[:])

                NTP = S // P // 2
                for qc in range(S // 512):
                    # attention, both heads, software-pipelined: AV lags one
                    # k-pair behind scores/sigmoid so PE never waits on ACT;
                    # out-proj blocks of the previous q-chunk fill PE slack.
                    pavs = {}
                    for h in range(HPC):
                        pavs[h] = ps2av.tile([P, 512], F32, tag="av", name=f"pav{h}")
                    prev_at = None
                    for tp in range(NTP):
                        ats = {}
                        for h in range(HPC):
                            pd = ps2d.tile([P, 1024], F32, tag="d", name="pd")
                            for u in range(2):
                                t = tp * 2 + u
                                nc.tensor.matmul(
                                    pd[:, u * 512:(u + 1) * 512],
                                    ksb[:, h, t * P:(t + 1) * P],
                                    qsb[:, h, qc * 512:(qc + 1) * 512],
                                    start=True, stop=True,
                                )
                            at = p2a.tile([P, 1024], DT16, tag="at", name="at")
                            nc.scalar.activation(
                                at[:], pd[:],
                                mybir.ActivationFunctionType.Sigmoid,
                                scale=1.0 / SCALE,
                            )
                            ats[h] = at
                        if prev_at is not None:
                            ptp, pats = prev_at
                            for h in range(HPC):
                                for u in range(2):
                                    t = ptp * 2 + u
                                    nc.tensor.matmul(
                                        pavs[h][:],
                                        v_sb[:, t, h, :, :].rearrange(
                                            "p b d -> p (b d)"),
                                        pats[h][:, u * 512:(u + 1) * 512],
                                        start=(t == 0), stop=False,
                                    )
                        if qc > 0 and tp < 8:
                            b, sq = divmod(tp, 4)
                            emit_out_block(b, (qc - 1) * 4 + sq)
                        prev_at = (tp, ats)
                    ptp, pats = prev_at
                    for h in range(HPC):
                        for u in range(2):
                            t = ptp * 2 + u
                            nc.tensor.matmul(
                                pavs[h][:],
                                v_sb[:, t, h, :, :].rearrange("p b d -> p (b d)"),
                                pats[h][:, u * 512:(u + 1) * 512],
                                start=False, stop=False,
                            )
                        nc.tensor.matmul(
                            pavs[h][HD:2 * HD, :], c1_sb[:, h, :], ones512[:],
                            start=False, stop=True,
                        )
                        for b in range(B):
                            nc.vector.tensor_copy(
                                vals_sb[h * HD:(h + 1) * HD, b,
                                        qc * 512:(qc + 1) * 512],
                                pavs[h][b * HD:(b + 1) * HD, :],
                            )
                # trailing out-proj blocks for the last q-chunk
                for b in range(B):
                    for sq in range(4):
                        emit_out_block(b, (S // 512 - 1) * 4 + sq, tail=True)

    nc.compile()
    return nc


_NC = None


def _get_nc():
    global _NC
    if _NC is None:
        _NC = build()
    return _NC


def kernel(x, w_q, w_k, w_v, W_o, _trace=False):
    x = np.asarray(x, dtype=np.float32)
    # host-side shard prep: transpose+cast x once, slice+cast weights per core
    x16t = np.ascontiguousarray(
        x.transpose(0, 2, 1).astype(np.float16))          # [B, D, S]
    w_q = np.asarray(w_q, dtype=np.float32)
    w_k = np.asarray(w_k, dtype=np.float32)
    w_v = np.asarray(w_v, dtype=np.float32)
    W_o = np.asarray(W_o, dtype=np.float32)

    nc = _get_nc()
    in_maps = []
    for i in range(NCORES):
        cs = slice(i * MS, (i + 1) * MS)
        in_maps.append({
            "xt": x16t,
            "wq": np.ascontiguousarray(w_q[:, cs].astype(np.float16)),
            "wk": np.ascontiguousarray(w_k[:, cs].astype(np.float16)),
            "wv": np.ascontiguousarray(w_v[:, cs].astype(np.float16)),
            "wo": np.ascontiguousarray(W_o[cs, :].astype(np.float16)),
        })
    try:
        res = bass_utils.run_bass_kernel_spmd(
            nc, in_maps, core_ids=list(range(NCORES)), trace=_trace
        )
    except Exception:
        # transient NRT exec failures have been observed to succeed on retry
        res = bass_utils.run_bass_kernel_spmd(
            nc, in_maps, core_ids=list(range(NCORES)), trace=_trace
        )
    out = res.results[0]["out"].astype(np.float32)
    for i in range(1, NCORES):
        out += res.results[i]["out"].astype(np.float32)
    out *= 1.0 / OSCALE
    if _trace:
        return out, res
    return out


# revision 33
# speedup vs baseline: 1.0115x; 1.0115x over previous
"""Trainium2 Bass kernel for nn_MHA_65429531787938.

MHA with a faithful-quirk softmax over dim=0 (the batch axis, B=2).
For B=2 the batch-softmax collapses to an elementwise sigmoid:
    attn0 = sigmoid((s0 - s1)/SCALE),  attn1 = 1 - attn0
and (1-A0) @ V1 = colsum(V1) - A0 @ V1, so a single attention matrix
serves both batches.

Sharding: tensor-parallel over the 16 heads -> 2 heads per core
(columns of w_q/w_k/w_v, rows of W_o). Each core consumes the full x
and produces a partial output (its heads' contribution to out = vals @ W_o,
scaled by 0.25 and stored fp16); the host sums the 8 partials and
multiplies by 4.

v2 changes vs the original baseline:
  - x is transposed + cast to fp16 on the HOST ([B, D, S]); the kernel
    DMA-loads x^T tiles directly, eliminating all 288 PE transposes,
    the 32 ACT fp32->fp16 casts, and half the x HBM traffic.
  - weights are cast to fp16 on the host (no on-chip weight casts).
  - output partials are written fp16 (scaled 0.25), halving write traffic.
  - Q/K psum->sbuf copies run on the Scalar engine (idle in phase 1),
    V copies on Vector, balancing the copy load.

Per-core pipeline (heads h0=2i, h1=2i+1 -> a 128-wide slice of q/k/v dims):
  phase 1: qT/kT/vT projections from DMA-loaded x^T chunks (fp16 matmuls,
           fp32 psum); qT/kT stored batch-stacked per head ([Q0;-Q1] /
           [K0;K1]); vT -> V natural via PE transpose (V1 stored negated).
  phase 2: d^T = K0@Q0^T - K1@Q1^T in one fused matmul (contraction=128);
           A0^T = sigmoid(d^T/SCALE) on ACT (fp16 out);
           psum_av = [V0 | -V1] @ A0^T + rank-1 colsum(V1) correction.
  phase 3: out_partial = 0.25 * vals @ W_o_slice (fp16 out).
"""

import numpy as np

import concourse.bacc as bacc
import concourse.mybir as mybir
import concourse.tile as tile
from concourse import bass_utils
from concourse.masks import make_identity

B, S, D, H = 2, 2048, 1024, 16
HD = 64
SCALE = float(D) ** 0.5
NCORES = 8
HPC = H // NCORES            # heads per core = 2
MS = HPC * HD                # per-core slice width = 128
P = 128
NCH = 8                      # phase-1 chunks (B * S/512)
DT16 = mybir.dt.float16
F32 = mybir.dt.float32
OSCALE = 0.25                # fp16 partial-output scale (host multiplies by 4)


def build():
    nc = bacc.Bacc("TRN2", target_bir_lowering=False, debug=False)

    # x arrives pre-transposed+cast on host: [B, D, S] fp16
    xt_d = nc.dram_tensor("xt", [B, D, S], DT16, kind="ExternalInput").ap()
    wq_d = nc.dram_tensor("wq", [D, MS], DT16, kind="ExternalInput").ap()
    wk_d = nc.dram_tensor("wk", [D, MS], DT16, kind="ExternalInput").ap()
    wv_d = nc.dram_tensor("wv", [D, MS], DT16, kind="ExternalInput").ap()
    wo_d = nc.dram_tensor("wo", [MS, D], DT16, kind="ExternalInput").ap()
    out_d = nc.dram_tensor("out", [B, S, D], DT16, kind="ExternalOutput").ap()

    with tile.TileContext(nc) as tc:
        with tc.tile_pool(name="persist", bufs=1) as pp:
            ident16 = pp.tile([P, P], DT16, name="ident16")
            make_identity(nc, ident16[:])
            ones512 = pp.tile([1, 512], DT16)
            nc.vector.memset(ones512[:], 1.0)
            ones128 = pp.tile([P, 1], DT16)
            nc.vector.memset(ones128[:], 1.0)

            # fp16 weights straight from DRAM
            w_sb = {}
            for name, dram in (("wq", wq_d), ("wk", wk_d), ("wv", wv_d)):
                t = pp.tile([P, D // P, MS], DT16, name=f"{name}_sb")
                nc.gpsimd.dma_start(t[:], dram.rearrange("(t p) m -> p t m", p=P))
                w_sb[name] = t
            wo_sb = pp.tile([P, 2, 512], DT16)
            nc.gpsimd.dma_start(
                wo_sb[:], wo_d.rearrange("p (c n) -> p c n", c=2)
            )

            # big persistent tensors
            qsb = pp.tile([P, HPC, S], DT16)     # [(b,hd), head, qpos], b1 negated
            ksb = pp.tile([P, HPC, S], DT16)     # [(b,hd), head, kpos]
            vt_sb = pp.tile([P, B, S], DT16)     # [(h,hd), batch, kpos], b1 negated
            v_sb = pp.tile([P, S // P, HPC, B, HD], DT16)  # [k, ktile, h, b, hd]
            vals_sb = pp.tile([P, B, S], DT16)   # [(h,hd), batch, qpos]
            c1_sb = pp.tile([1, HPC, HD], DT16)  # +colsum(V1) per head

            # ---------------- phase 1: Q/K/V projections ----------------
            with tc.tile_pool(name="p1xt", bufs=4) as p1xt, \
                 tc.tile_pool(name="ps1", bufs=4, space="PSUM") as ps1, \
                 tc.tile_pool(name="ps1v", bufs=2, space="PSUM") as ps1v, \
                 tc.tile_pool(name="psW", bufs=1, space="PSUM") as psW:
                # PE warm-up: the first real matmul waits ~15us for the cold
                # DMA queues, long enough for the HAM clock gate to throttle
                # PE to 1.2GHz. A chain of dummy transposes (WAW-serialized,
                # zero real cost while PE would idle) keeps the activity
                # window busy; the second batch reads the first x chunk so it
                # always bridges right up to the first projection.
                wt = psW.tile([P, P], DT16, tag="w", name="wt")
                for _ in range(60):
                    nc.tensor.transpose(wt[:], ident16[:], ident16[:])
                for c in range(NCH):
                    b, j = divmod(c, NCH // B)
                    xt = p1xt.tile([P, D // P, 512], DT16, tag="xt")
                    dma_eng = nc.sync if c % 2 == 0 else nc.gpsimd
                    if c == 0:
                        # split the very first load across both rings so the
                        # pipeline fills faster
                        src = xt_d[b, :, j * 512:(j + 1) * 512].rearrange(
                            "(t p) s -> p t s", p=P)
                        nc.sync.dma_start(xt[:, :4, :], src[:, :4, :])
                        nc.gpsimd.dma_start(xt[:, 4:, :], src[:, 4:, :])
                        for _ in range(30):
                            nc.tensor.transpose(wt[:], xt[:, 0, 0:P],
                                                ident16[:])
                    else:
                        dma_eng.dma_start(
                            xt[:],
                            xt_d[b, :, j * 512:(j + 1) * 512].rearrange(
                                "(t p) s -> p t s", p=P),
                        )
                    # Q/K projections -> ACT copies (batch-stacked layout)
                    for name, dest, neg in (("wq", qsb, True), ("wk", ksb, False)):
                        ps = ps1.tile([P, 512], F32, tag="proj", name="ps_p")
                        for t in range(D // P):
                            nc.tensor.matmul(
                                ps[:], w_sb[name][:, t, :], xt[:, t, :],
                                start=(t == 0), stop=(t == D // P - 1),
                            )
                        for h in range(HPC):
                            nc.scalar.mul(
                                dest[b * HD:(b + 1) * HD, h, j * 512:(j + 1) * 512],
                                ps[h * HD:(h + 1) * HD, :],
                                -1.0 if (neg and b == 1) else 1.0,
                            )
                    # V projection -> DVE copy (vT, b1 negated)
                    ps = ps1.tile([P, 512], F32, tag="proj", name="ps_p")
                    for t in range(D // P):
                        nc.tensor.matmul(
                            ps[:], w_sb["wv"][:, t, :], xt[:, t, :],
                            start=(t == 0), stop=(t == D // P - 1),
                        )
                    nc.vector.tensor_scalar_mul(
                        vt_sb[:, b, j * 512:(j + 1) * 512], ps[:],
                        -1.0 if b == 1 else 1.0,
                    )
                    # V natural layout for the 4 k-tiles of this chunk
                    pvt = ps1v.tile([P, 4, P], DT16, tag="vt", name="pvt")
                    for blk in range(4):
                        t = j * 4 + blk
                        nc.tensor.transpose(
                            pvt[:, blk, :], vt_sb[:, b, t * P:(t + 1) * P],
                            ident16[:],
                        )
                    nc.vector.tensor_copy(
                        v_sb[:, j * 4:(j + 1) * 4, :, b, :],
                        pvt[:].rearrange("p t (h d) -> p t h d", h=HPC),
                    )

            # ------------- phase 2 + 3, interleaved -------------
            with tc.tile_pool(name="p2a", bufs=8) as p2a, \
                 tc.tile_pool(name="p3o", bufs=4) as p3o, \
                 tc.tile_pool(name="ps2d", bufs=2, space="PSUM") as ps2d, \
                 tc.tile_pool(name="ps2av", bufs=2, space="PSUM") as ps2av, \
                 tc.tile_pool(name="ps3", bufs=2, space="PSUM") as ps3:
                # colsums of all (h, b) V columns; extract b=1 (stored negated)
                pc1 = ps3.tile([1, HPC * B * HD], F32, tag="o")
                for t in range(S // P):
                    nc.tensor.matmul(
                        pc1[:], ones128[:],
                        v_sb[:, t, :, :, :].rearrange("p h b d -> p (h b d)"),
                        start=(t == 0), stop=(t == S // P - 1),
                    )
                nc.vector.tensor_scalar_mul(
                    c1_sb[:],
                    pc1[:].rearrange("p (h b d) -> p h b d", h=HPC, b=B)[:, :, 1, :],
                    -1.0,
                )

                def emit_out_block(b, si, tail=False):
                    # one output-projection s-block (phase 3); fp16 out, 0.25x
                    ot = p3o.tile([P, D], DT16, tag="ot", name="ot")
                    if tail and si % 2 == 0:
                        # scores psum (pd ring) is idle in the tail: even
                        # blocks borrow a [P,1024] tile with one big ACT
                        # copy; odd blocks use the ps3 ring with DVE copies,
                        # so 4 blocks pipeline across disjoint psum + engines
                        po = ps2d.tile([P, 1024], F32, tag="d", name="pd")
                        for nch in range(2):
                            nc.tensor.matmul(
                                po[:, nch * 512:(nch + 1) * 512],
                                vals_sb[:, b, si * P:(si + 1) * P],
                                wo_sb[:, nch, :],
                                start=True, stop=True,
                            )
                        nc.scalar.mul(ot[:], po[:], OSCALE)
                        ring = nc.sync
                    elif tail:
                        for nch in range(2):
                            po = ps3.tile([P, 512], F32, tag="o", name="po")
                            nc.tensor.matmul(
                                po[:],
                                vals_sb[:, b, si * P:(si + 1) * P],
                                wo_sb[:, nch, :],
                                start=True, stop=True,
                            )
                            nc.vector.tensor_scalar_mul(
                                ot[:, nch * 512:(nch + 1) * 512], po[:], OSCALE
                            )
                        ring = nc.gpsimd
                    else:
                        for nch in range(2):
                            po = ps3.tile([P, 512], F32, tag="o", name="po")
                            nc.tensor.matmul(
                                po[:],
                                vals_sb[:, b, si * P:(si + 1) * P],
                                wo_sb[:, nch, :],
                                start=True, stop=True,
                            )
                            nc.vector.tensor_scalar_mul(
                                ot[:, nch * 512:(nch + 1) * 512], po[:], OSCALE
                            )
                        ring = nc.sync
                    ring.dma_start(out_d[b, si * P:(si + 1) * P, :], ot[:])

                NTP = S // P // 2
                for qc in range(S // 512):
                    # attention, both heads, software-pipelined: AV lags one
                    # k-pair behind scores/sigmoid so PE never waits on ACT;
                    # out-proj blocks of the previous q-chunk fill PE slack.
                    pavs = {}
                    for h in range(HPC):
                        pavs[h] = ps2av.tile([P, 512], F32, tag="av", name=f"pav{h}")
                    prev_at = None
                    for tp in range(NTP):
                        ats = {}
                        for h in range(HPC):
                            pd = ps2d.tile([P, 1024], F32, tag="d", name="pd")
                            for u in range(2):
                                t = tp * 2 + u
                                nc.tensor.matmul(
                                    pd[:, u * 512:(u + 1) * 512],
                                    ksb[:, h, t * P:(t + 1) * P],
                                    qsb[:, h, qc * 512:(qc + 1) * 512],
                                    start=True, stop=True,
                                )
                            at = p2a.tile([P, 1024], DT16, tag="at", name="at")
                            nc.scalar.activation(
                                at[:], pd[:],
                                mybir.ActivationFunctionType.Sigmoid,
                                scale=1.0 / SCALE,
                            )
                            ats[h] = at
                        if prev_at is not None:
                            ptp, pats = prev_at
                            for h in range(HPC):
                                for u in range(2):
                                    t = ptp * 2 + u
                                    nc.tensor.matmul(
                                        pavs[h][:],
                                        v_sb[:, t, h, :, :].rearrange(
                                            "p b d -> p (b d)"),
                                        pats[h][:, u * 512:(u + 1) * 512],
                                        start=(t == 0), stop=False,
                                    )
                        if qc > 0 and tp < 8:
                            b, sq = divmod(tp, 4)
                            emit_out_block(b, (qc - 1) * 4 + sq)
                        prev_at = (tp, ats)
                    ptp, pats = prev_at
                    for h in range(HPC):
                        for u in range(2):
                            t = ptp * 2 + u
                            nc.tensor.matmul(
                                pavs[h][:],
                                v_sb[:, t, h, :, :].rearrange("p b d -> p (b d)"),
                                pats[h][:, u * 512:(u + 1) * 512],
                                start=False, stop=False,
                            )
                        nc.tensor.matmul(
                            pavs[h][HD:2 * HD, :], c1_sb[:, h, :], ones512[:],
                            start=False, stop=True,
                        )
                        for b in range(B):
                            nc.vector.tensor_copy(
                                vals_sb[h * HD:(h + 1) * HD, b,
                                        qc * 512:(qc + 1) * 512],
                                pavs[h][b * HD:(b + 1) * HD, :],
                            )
                # trailing out-proj blocks for the last q-chunk
                for b in range(B):
                    for sq in range(4):
                        emit_out_block(b, (S // 512 - 1) * 4 + sq, tail=True)

    nc.compile()
    return nc


_NC = None


def _get_nc():
    global _NC
    if _NC is None:
        _NC = build()
    return _NC


def kernel(x, w_q, w_k, w_v, W_o, _trace=False):
    x = np.asarray(x, dtype=np.float32)
    # host-side shard prep: transpose+cast x once, slice+cast weights per core
    x16t = np.ascontiguousarray(
        x.transpose(0, 2, 1).astype(np.float16))          # [B, D, S]
    w_q = np.asarray(w_q, dtype=np.float32)
    w_k = np.asarray(w_k, dtype=np.float32)
    w_v = np.asarray(w_v, dtype=np.float32)
    W_o = np.asarray(W_o, dtype=np.float32)

    nc = _get_nc()
    in_maps = []
    for i in range(NCORES):
        cs = slice(i * MS, (i + 1) * MS)
        in_maps.append({
            "xt": x16t,
            "wq": np.ascontiguousarray(w_q[:, cs].astype(np.float16)),
            "wk": np.ascontiguousarray(w_k[:, cs].astype(np.float16)),
            "wv": np.ascontiguousarray(w_v[:, cs].astype(np.float16)),
            "wo": np.ascontiguousarray(W_o[cs, :].astype(np.float16)),
        })
    try:
        res = bass_utils.run_bass_kernel_spmd(
            nc, in_maps, core_ids=list(range(NCORES)), trace=_trace
        )
    except Exception:
        # transient NRT exec failures have been observed to succeed on retry
        res = bass_utils.run_bass_kernel_spmd(
            nc, in_maps, core_ids=list(range(NCORES)), trace=_trace
        )
    out = res.results[0]["out"].astype(np.float32)
    for i in range(1, NCORES):
        out += res.results[i]["out"].astype(np.float32)
    out *= 1.0 / OSCALE
    if _trace:
        return out, res
    return out
